# revision 1
# baseline (speedup 1.0000x reference)
"""Trainium2 Bass kernel for the CT-metrics pairwise MLP scorer.

Computes, for M_aug [N,D] and Q [M,D] (N=M=512, D=1024):
    diff2[n,m,:] = (M_aug[n]-Q[m])**2
    cost[n,m]    = diff2.sum(-1)
    d[n,m]       = -(MLP(diff2[n,m,:]) + b3)      (D->512->256->1, leaky relu)
    bw           = softmax(d, axis=0)
    score[m]     = sum_n cost*bw,  score_fg[m] = sum_{n<N_fg} cost*bw

Sharding: N axis split across 8 cores (64 rows each). Each core computes
partial column sums S1 = sum_n exp(d), S1fg, Sc' = sum_n exp(d)*(||Mn||^2
- 2 M.Q^T) and Scfg' (flash-softmax style; logits are O(1) so no max shift
is needed).  Host combine:  score = Sc'/S1 + ||Q||^2,
score_fg = Scfg'/S1 + ||Q||^2 * S1fg/S1.

Device layout: contraction dims sit on SBUF partitions.  Per local row n:
diff2^T[d, m] chunk tiles are produced on THREE engines (ACT fused
square-with-bias; DVE and GPSIMD as add-broadcast + self-multiply) so no
single elementwise engine bottlenecks; layers 1/2/3 are bf16 matmuls
(fp32 PSUM accumulate); layer 3 uses h2 slices as stationary so the
logits land transposed as [m_chunk, n] in PSUM, giving free-dim softmax
reductions.  Inputs arrive packed in two DRAM tensors (one f32, one bf16)
so the whole load is 2 DMAs; output per core is a [128, 4, 4] stats tile.
"""

from contextlib import ExitStack

import numpy as np

import concourse.bass as bass
import concourse.tile as tile
from concourse import bacc, mybir
from concourse.bass_utils import run_bass_kernel_spmd

N_CORES = 8
N, M, D, H = 512, 512, 1024, 512
K2 = H // 2  # 256
NL = N // N_CORES  # 64 rows per core
DC, HC, KC, MC = D // 128, H // 128, K2 // 128, M // 128  # 8, 4, 2, 4
NEG_SLOPE = 0.01

F32 = mybir.dt.float32
BF16 = mybir.dt.bfloat16
AF = mybir.ActivationFunctionType

# d2 chunk producer per dc index: 'a' = ACT fused square,
# 'v' = DVE 2-step, 'p' = GPSIMD 2-step
D2_SPLIT = "aaaaaaaa"

# packed f32 column offsets
_QT0 = 0
_MT0 = _QT0 + DC * M          # 4096
_B10 = _MT0 + DC * NL         # 4608
_B20 = _B10 + HC              # 4612
_B30 = _B20 + KC              # 4614
_MN0 = _B30 + 1               # 4615  (0.5*||Mn||^2 row, partition 0)
_FG0 = _MN0 + NL              # 4679  (fg mask row tiled MC times, part 0)
_ON0 = _FG0 + MC * NL         # 4935  (ones row, partition 0)
_PF_COLS = _ON0 + 128         # 5063

# packed bf16 column offsets
_W10 = 0
_W20 = _W10 + DC * H          # 4096
_W30 = _W20 + HC * K2         # 5120
_PB_COLS = _W30 + KC          # 5122


def emit_body(nc, tc, ctx, pf_sb, pb_sb, stats, act_fn, d2_split=D2_SPLIT,
              pools=None, fake_d2=False, batch2=False):
    """Emit prologue + n-loop + epilogue reading packed SBUF tiles."""
    if pools is None:
        pools = {}

    def pool(name, bufs, space="SBUF"):
        if name not in pools:
            pools[name] = ctx.enter_context(
                tc.tile_pool(name=name, bufs=bufs, space=space))
        return pools[name]

    consts = pool("consts", 1)
    diffp = pool("diffp", 2)
    h1p = pool("h1p", 2)
    h2p = pool("h2p", 2)
    ep = pool("ep", 1)
    tvp = pool("tvp", 2)
    tpp = pool("tpp", 2)
    l1ps = pool("l1ps", 3 if batch2 else 5, "PSUM")
    l2ps = pool("l2ps", 2 if batch2 else 1, "PSUM")
    l3ps = pool("l3ps", 1, "PSUM")

    qt_sb = pf_sb[:, _QT0:_MT0].rearrange("p (c m) -> p c m", c=DC)
    mt_sb = pf_sb[:, _MT0:_B10].rearrange("p (c n) -> p c n", c=DC)
    b1_sb = pf_sb[:, _B10:_B20]
    b2_sb = pf_sb[:, _B20:_B30]
    b3_sb = pf_sb[:, _B30:_B30 + 1]
    mn2h = pf_sb[0:1, _MN0:_MN0 + NL]
    fg_row = pf_sb[0:1, _FG0:_FG0 + MC * NL]
    onesr = pf_sb[0:1, _ON0:_ON0 + 128]
    w1_sb = pb_sb[:, _W10:_W20].rearrange("p (c h) -> p c h", c=DC)
    w2_sb = pb_sb[:, _W20:_W30].rearrange("p (c k) -> p c k", c=HC)
    w3_sb = pb_sb[:, _W30:_W30 + KC].rearrange("p (c o) -> p c o", c=KC)

    # ---- prologue (all deps resolve to the two input DMAs) ----
    g_t = l3ps.tile([128, MC, NL], F32, tag="ps3")
    for mc in range(MC):
        for dc in range(DC):
            nc.tensor.matmul(g_t[:, mc, :],
                             qt_sb[:, dc, mc * 128:(mc + 1) * 128],
                             mt_sb[:, dc, :],
                             start=(dc == 0), stop=False)
        nc.tensor.matmul(g_t[:, mc, :], onesr, mn2h, start=False, stop=True)
    cost_t = consts.tile([128, MC, NL], F32, tag="cost_t")
    nc.vector.tensor_scalar_mul(cost_t[:], g_t[:], 2.0)

    mask_ps = l1ps.tile([128, MC * NL], F32, tag="ps1")
    nc.tensor.matmul(mask_ps[:], onesr, fg_row, start=True, stop=True)
    mask_bc = consts.tile([128, MC, NL], F32, tag="mask_bc")
    nc.vector.tensor_copy(mask_bc[:], mask_ps[:].rearrange(
        "p (c n) -> p c n", c=MC))

    d_ps = l3ps.tile([128, MC, NL], F32, tag="ps3")

    def produce_d2(n):
        if fake_d2:
            return pb_sb[:, 0:DC * M].rearrange("p (c m) -> p c m", c=DC)
        d2 = diffp.tile([128, DC, M], BF16, tag="d2", name="d2")
        for dc in range(DC):
            eng = d2_split[dc]
            if eng == "a":
                nc.scalar.activation(d2[:, dc, :], qt_sb[:, dc, :], AF.Square,
                                     bias=mt_sb[:, dc, n:n + 1])
            elif eng == "v":
                tv = tvp.tile([128, M], BF16, tag="tv", name="tv")
                nc.vector.tensor_scalar_add(tv[:], qt_sb[:, dc, :],
                                            mt_sb[:, dc, n:n + 1])
                nc.vector.tensor_mul(d2[:, dc, :], tv[:], tv[:])
            else:
                tp = tpp.tile([128, M], BF16, tag="tp", name="tp")
                nc.gpsimd.tensor_scalar_add(tp[:], qt_sb[:, dc, :],
                                            mt_sb[:, dc, n:n + 1])
                nc.gpsimd.tensor_mul(d2[:, dc, :], tp[:], tp[:])
        return d2

    def l3_emit(n, h2):
        for mc in range(MC):
            for kc in range(KC):
                nc.tensor.matmul(
                    d_ps[:, mc, n:n + 1],
                    h2[:, kc, mc * 128:(mc + 1) * 128],
                    w3_sb[:, kc, :],
                    start=(kc == 0), stop=(kc == KC - 1))

    if batch2:
        for n in range(0, NL, 2):
            d2a = produce_d2(n)
            d2b = produce_d2(n + 1)
            h1a = h1p.tile([128, HC, M], BF16, tag="h1", name="h1a")
            h1b = h1p.tile([128, HC, M], BF16, tag="h1", name="h1b")
            for hc in range(HC):
                p1a = l1ps.tile([128, M], F32, tag="ps1", name="p1a")
                p1b = l1ps.tile([128, M], F32, tag="ps1", name="p1b")
                for dc in range(DC):
                    w_slice = w1_sb[:, dc, hc * 128:(hc + 1) * 128]
                    nc.tensor.matmul(p1a[:], w_slice, d2a[:, dc, :],
                                     start=(dc == 0), stop=(dc == DC - 1))
                    nc.tensor.matmul(p1b[:], w_slice, d2b[:, dc, :],
                                     start=(dc == 0), stop=(dc == DC - 1))
                nc.scalar.activation(h1a[:, hc, :], p1a[:], act_fn,
                                     bias=b1_sb[:, hc:hc + 1],
                                     alpha=NEG_SLOPE)
                nc.scalar.activation(h1b[:, hc, :], p1b[:], act_fn,
                                     bias=b1_sb[:, hc:hc + 1],
                                     alpha=NEG_SLOPE)
            p2a = l2ps.tile([128, KC, M], F32, tag="ps2", name="p2a")
            p2b = l2ps.tile([128, KC, M], F32, tag="ps2", name="p2b")
            for kc in range(KC):
                for hc in range(HC):
                    w_slice = w2_sb[:, hc, kc * 128:(kc + 1) * 128]
                    nc.tensor.matmul(p2a[:, kc, :], w_slice, h1a[:, hc, :],
                                     start=(hc == 0), stop=(hc == HC - 1))
                    nc.tensor.matmul(p2b[:, kc, :], w_slice, h1b[:, hc, :],
                                     start=(hc == 0), stop=(hc == HC - 1))
            h2a = h2p.tile([128, KC, M], BF16, tag="h2", name="h2a")
            h2b = h2p.tile([128, KC, M], BF16, tag="h2", name="h2b")
            for kc in range(KC):
                nc.scalar.activation(h2a[:, kc, :], p2a[:, kc, :], act_fn,
                                     bias=b2_sb[:, kc:kc + 1],
                                     alpha=NEG_SLOPE)
                nc.scalar.activation(h2b[:, kc, :], p2b[:, kc, :], act_fn,
                                     bias=b2_sb[:, kc:kc + 1],
                                     alpha=NEG_SLOPE)
            l3_emit(n, h2a)
            l3_emit(n + 1, h2b)
    else:
        _unused = 0
    # ---- main loop over local rows ----
    for n in range(NL if not batch2 else 0):
        if fake_d2:
            d2 = pb_sb[:, 0:DC * M].rearrange("p (c m) -> p c m", c=DC)
        else:
            d2 = diffp.tile([128, DC, M], BF16, tag="d2")
        for dc in range(DC if not fake_d2 else 0):
            eng = d2_split[dc]
            if eng == "a":
                nc.scalar.activation(d2[:, dc, :], qt_sb[:, dc, :], AF.Square,
                                     bias=mt_sb[:, dc, n:n + 1])
            elif eng == "v":
                tv = tvp.tile([128, M], BF16, tag="tv")
                nc.vector.tensor_scalar_add(tv[:], qt_sb[:, dc, :],
                                            mt_sb[:, dc, n:n + 1])
                nc.vector.tensor_mul(d2[:, dc, :], tv[:], tv[:])
            else:
                tp = tpp.tile([128, M], BF16, tag="tp")
                nc.gpsimd.tensor_scalar_add(tp[:], qt_sb[:, dc, :],
                                            mt_sb[:, dc, n:n + 1])
                nc.gpsimd.tensor_mul(d2[:, dc, :], tp[:], tp[:])

        h1 = h1p.tile([128, HC, M], BF16, tag="h1")
        for hc in range(HC):
            p1 = l1ps.tile([128, M], F32, tag="ps1")
            for dc in range(DC):
                nc.tensor.matmul(
                    p1[:],
                    w1_sb[:, dc, hc * 128:(hc + 1) * 128],
                    d2[:, dc, :],
                    start=(dc == 0), stop=(dc == DC - 1))
            nc.scalar.activation(h1[:, hc, :], p1[:], act_fn,
                                 bias=b1_sb[:, hc:hc + 1], alpha=NEG_SLOPE)

        p2 = l2ps.tile([128, KC, M], F32, tag="ps2")
        for kc in range(KC):
            for hc in range(HC):
                nc.tensor.matmul(
                    p2[:, kc, :],
                    w2_sb[:, hc, kc * 128:(kc + 1) * 128],
                    h1[:, hc, :],
                    start=(hc == 0), stop=(hc == HC - 1))
        h2 = h2p.tile([128, KC, M], BF16, tag="h2")
        for kc in range(KC):
            nc.scalar.activation(h2[:, kc, :], p2[:, kc, :], act_fn,
                                 bias=b2_sb[:, kc:kc + 1], alpha=NEG_SLOPE)

        for mc in range(MC):
            for kc in range(KC):
                nc.tensor.matmul(
                    d_ps[:, mc, n:n + 1],
                    h2[:, kc, mc * 128:(mc + 1) * 128],
                    w3_sb[:, kc, :],
                    start=(kc == 0), stop=(kc == KC - 1))

    # ---- epilogue ----
    e_t = ep.tile([128, MC, NL], F32, tag="e_t")
    nc.scalar.activation(e_t[:], d_ps[:], AF.Exp, bias=b3_sb, scale=-1.0)
    w_t = ep.tile([128, MC, NL], F32, tag="w_t")
    nc.vector.tensor_mul(w_t[:], e_t[:], cost_t[:])
    efg_t = ep.tile([128, MC, NL], F32, tag="efg_t")
    nc.vector.tensor_mul(efg_t[:], e_t[:], mask_bc[:])
    wfg_t = ep.tile([128, MC, NL], F32, tag="wfg_t")
    nc.vector.tensor_mul(wfg_t[:], w_t[:], mask_bc[:])

    stats_sb = consts.tile([128, 4, MC], F32, tag="stats_sb")
    for s, src in ((0, e_t), (1, efg_t), (2, w_t), (3, wfg_t)):
        nc.vector.tensor_reduce(stats_sb[:, s, :], src[:],
                                axis=mybir.AxisListType.X,
                                op=mybir.AluOpType.add)
    nc.sync.dma_start(stats[:], stats_sb[:])


def build_program(act_fn=AF.Lrelu, d2_split=D2_SPLIT):
    nc = bacc.Bacc("TRN2", target_bir_lowering=False, debug=False,
                   num_devices=N_CORES)
    pf = nc.dram_tensor("pf", [128, _PF_COLS], F32, kind="ExternalInput").ap()
    pb = nc.dram_tensor("pb", [128, _PB_COLS], BF16, kind="ExternalInput").ap()
    stats = nc.dram_tensor("stats", [128, 4, MC], F32,
                           kind="ExternalOutput").ap()

    with tile.TileContext(nc) as tc, ExitStack() as ctx:
        consts = ctx.enter_context(tc.tile_pool(name="consts", bufs=1))
        pf_sb = consts.tile([128, _PF_COLS], F32, tag="pf_sb")
        nc.sync.dma_start(pf_sb[:], pf[:])
        pb_sb = consts.tile([128, _PB_COLS], BF16, tag="pb_sb")
        nc.sync.dma_start(pb_sb[:], pb[:])
        emit_body(nc, tc, ctx, pf_sb, pb_sb, stats, act_fn, d2_split,
                  pools={"consts": consts})

    nc.compile()
    return nc


def shard_inputs(M_aug, Q, W1, b1, W2, b2, W3, b3, N_fg):
    """Host-side layout prep. Returns per-core input maps."""
    import ml_dtypes
    f = np.float32
    bf = ml_dtypes.bfloat16
    M_aug = np.asarray(M_aug, f)
    Q = np.asarray(Q, f)
    W1 = np.asarray(W1, f)
    W2 = np.asarray(W2, f)
    W3 = np.asarray(W3, f)
    b1 = np.asarray(b1, f)
    b2 = np.asarray(b2, f)
    b3 = np.asarray(b3, f)
    nfg = int(N_fg)

    def part_major(a2d, chunks):  # [C*128, F] -> [128, C*F]
        cdim, fdim = a2d.shape
        assert cdim == chunks * 128
        return np.ascontiguousarray(
            a2d.reshape(chunks, 128, fdim).transpose(1, 0, 2)).reshape(128, -1)

    pb_v = np.zeros((128, _PB_COLS), bf)
    pb_v[:, _W10:_W20] = part_major(W1.T, DC).astype(bf)
    pb_v[:, _W20:_W30] = part_major(W2.T, HC).astype(bf)
    pb_v[:, _W30:_W30 + KC] = part_major(W3.reshape(K2, 1), KC).astype(bf)
    pb_v = np.ascontiguousarray(pb_v)

    base = np.zeros((128, _PF_COLS), f)
    base[:, _QT0:_MT0] = part_major(Q.T, DC)
    base[:, _B10:_B20] = b1.reshape(HC, 128).T
    base[:, _B20:_B30] = b2.reshape(KC, 128).T
    base[:, _B30] = -float(b3[0])
    base[0, _ON0:_ON0 + 128] = 1.0

    in_maps = []
    for c in range(N_CORES):
        rows = slice(c * NL, (c + 1) * NL)
        pf_v = base.copy()
        pf_v[:, _MT0:_B10] = part_major(-M_aug[rows].T, DC)
        pf_v[0, _MN0:_MN0 + NL] = 0.5 * (M_aug[rows] ** 2).sum(-1)
        gidx = np.arange(c * NL, (c + 1) * NL)
        pf_v[0, _FG0:_FG0 + MC * NL] = np.tile((gidx < nfg).astype(f), MC)
        in_maps.append({"pf": pf_v, "pb": pb_v})
    return in_maps


def combine(stats_list, Q):
    """stats_list: per-core [128, 4, MC] arrays -> (score, score_fg)."""
    st = np.stack([
        np.asarray(s, np.float64).transpose(1, 2, 0).reshape(4, M)
        for s in stats_list
    ])  # [C, 4, M]
    S1 = st[:, 0].sum(0)
    S1fg = st[:, 1].sum(0)
    Sc = st[:, 2].sum(0)
    Scfg = st[:, 3].sum(0)
    qn2 = (np.asarray(Q, np.float64) ** 2).sum(-1)
    score = Sc / S1 + qn2
    score_fg = Scfg / S1 + qn2 * (S1fg / S1)
    return score.astype(np.float32), score_fg.astype(np.float32)


_PROGRAM_CACHE = {}


def run(trace=False, **inputs):
    if "prog" not in _PROGRAM_CACHE:
        _PROGRAM_CACHE["prog"] = build_program()
    nc = _PROGRAM_CACHE["prog"]
    in_maps = shard_inputs(**inputs)
    res = run_bass_kernel_spmd(nc, in_maps, list(range(N_CORES)), trace=trace)
    outs = combine([res.results[c]["stats"] for c in range(N_CORES)],
                   inputs["Q"])
    return outs, res


def kernel(**inputs):
    outs, _ = run(trace=False, **inputs)
    return outs



# revision 2
# speedup vs baseline: 1.9932x; 1.9932x over previous
"""Trainium2 Bass kernel for the CT-metrics pairwise MLP scorer.

Computes, for M_aug [N,D] and Q [M,D] (N=M=512, D=1024):
    diff2[n,m,:] = (M_aug[n]-Q[m])**2
    cost[n,m]    = diff2.sum(-1)
    d[n,m]       = -(MLP(diff2[n,m,:]) + b3)      (D->512->256->1, leaky relu)
    bw           = softmax(d, axis=0)
    score[m]     = sum_n cost*bw,  score_fg[m] = sum_{n<N_fg} cost*bw

Sharding: N axis split across 8 cores (64 rows each). Each core computes
partial column sums S1 = sum_n exp(d), S1fg, Sc' = sum_n exp(d)*(||Mn||^2
- 2 M.Q^T) and Scfg' (flash-softmax; logits are O(1) so no max shift).
Host combine:  score = Sc'/S1 + ||Q||^2, score_fg = Scfg'/S1 + ||Q||^2*S1fg/S1.

Device algorithm (fp8 DoubleRow formulation):
  diff2 = q^2 - 2mq + m^2.  The n-independent W1 @ q^2 term is folded into
  a precomputed Btilde[h,m] = W1 @ q^2 + b1 (computed once on device from
  host-packed q^2, stored fp8, injected into each row's L1 PSUM via an
  identity-pair DoubleRow matmul).  Per row only
      u[d,m] = q[d,m]*(-2 m[n,d]) + m[n,d]^2
  is materialized - a single DVE/GPSIMD tensor_scalar op (bf16 in, f32
  scalars, fp8 out; DVE runs it in 2x_2p mode).  All three MLP layers are
  fp8e4 DoubleRow matmuls (0.5 cyc/row, 256-contraction per instruction);
  b2 is injected as an fp8 rank-2 DoubleRow matmul; b1 lives in Btilde, so
  the h1/h2 activations are bias-free and read multi-bank PSUM pairs in
  single ACT instructions.  Engine balance per row: ACT ~3.1us (h1+h2),
  PE ~2.7us (matmuls), DVE ~2.0us (6 u-chunks), GPSIMD ~1.6us (2 u-chunks).
"""

from contextlib import ExitStack

import numpy as np

import concourse.bass as bass
import concourse.tile as tile
from concourse import bacc, mybir
from concourse.bass_utils import run_bass_kernel_spmd

N_CORES = 8
N, M, D, H = 512, 512, 1024, 512
K2 = H // 2  # 256
NL = N // N_CORES  # 64 rows per core
DC, HC, KC, MC = D // 128, H // 128, K2 // 128, M // 128  # 8, 4, 2, 4
DP, HP = DC // 2, HC // 2  # DoubleRow plane-pairs: 4, 2
NEG_SLOPE = 0.01

F32 = mybir.dt.float32
BF16 = mybir.dt.bfloat16
FP8 = mybir.dt.float8e4
AF = mybir.ActivationFunctionType
DR = mybir.MatmulPerfMode.DoubleRow

# which engine produces each u chunk: 'v' = DVE, 'p' = GPSIMD
U_SPLIT = "vvpvvpvv"

# ---- packed f32 tensor column offsets ----
_S10 = 0                       # -2*M^T  [128, DC*NL]
_S20 = _S10 + DC * NL          # (M^T)^2 [128, DC*NL]
_MN0 = _S20 + DC * NL          # 0.5*||Mn||^2 row (partition 0) [1, NL]
_FG0 = _MN0 + NL               # fg mask row tiled MC times (part 0)
_ON0 = _FG0 + MC * NL          # ones row f32 (part 0) [1, 128]
_B30 = _ON0 + 128              # -b3 column [128, 1]
_PF_COLS = _B30 + 1

# ---- packed bf16 tensor column offsets ----
_QT0 = 0                       # Q^T chunks [128, DC*M]
_MT0 = _QT0 + DC * M           # -M^T chunks [128, DC*NL]
_B10 = _MT0 + DC * NL          # b1 row (part 0) [1, H]
_OB0 = _B10 + H                # ones row bf16 (part 0) [1, M]
_PB_COLS = _OB0 + M

# ---- packed fp8 tensor column offsets ----
_W10 = 0                       # W1 DoubleRow layout [128, DP*2*H]
_W20 = _W10 + DP * 2 * H       # W2 DoubleRow layout [128, HP*2*K2]
_W30 = _W20 + HP * 2 * K2      # W3 DoubleRow layout [128, 2*1]
_Q20 = _W30 + 2                # (Q^T)^2 DoubleRow layout [128, DP*2*M]
_IH0 = _Q20 + DP * 2 * M       # 0.5*I identity pair [128, 2*128]
_BB0 = _IH0 + 2 * 128          # b2/2 pairs (part 0) [1, KC*2*128]
_O80 = _BB0 + KC * 2 * 128     # ones pair fp8 (part 0) [1, 2*M]
_P8_COLS = _O80 + 2 * M


def emit_body(nc, tc, ctx, pf_sb, pb_sb, p8_sb, stats):
    def pool(name, bufs, space="SBUF"):
        return ctx.enter_context(tc.tile_pool(name=name, bufs=bufs, space=space))

    consts = ctx.enter_context(tc.tile_pool(name="consts2", bufs=1))
    diffp = pool("diffp", 2)
    h1p = pool("h1p", 2)
    h2p = pool("h2p", 2)
    ep = pool("ep", 1)
    # one shared 2-bank PSUM pool: per row holds l1a, l1b, p2 (rotating)
    ps2b = pool("ps2b", 3, "PSUM")
    psd = pool("psd", 1, "PSUM")  # [128, MC, NL] logits / cost psum

    s1f = pf_sb[:, _S10:_S20].rearrange("p (c n) -> p c n", c=DC)
    s2f = pf_sb[:, _S20:_MN0].rearrange("p (c n) -> p c n", c=DC)
    mn2h = pf_sb[0:1, _MN0:_MN0 + NL]
    fg_row = pf_sb[0:1, _FG0:_FG0 + MC * NL]
    onesf = pf_sb[0:1, _ON0:_ON0 + 128]
    b3n = pf_sb[:, _B30:_B30 + 1]

    qt_bf = pb_sb[:, _QT0:_MT0].rearrange("p (c m) -> p c m", c=DC)
    mt_bf = pb_sb[:, _MT0:_B10].rearrange("p (c n) -> p c n", c=DC)
    b1_bf = pb_sb[0:1, _B10:_B10 + H]
    onesb = pb_sb[0:1, _OB0:_OB0 + M]

    w1_8 = p8_sb[:, _W10:_W20].rearrange("p (i j h) -> p i j h", i=DP, j=2)
    w2_8 = p8_sb[:, _W20:_W30].rearrange("p (i j k) -> p i j k", i=HP, j=2)
    w3_8 = p8_sb[:, _W30:_W30 + 2].rearrange("p (j o) -> p j o", j=2)
    q2_8 = p8_sb[:, _Q20:_IH0].rearrange("p (i j m) -> p i j m", i=DP, j=2)
    ih_8 = p8_sb[:, _IH0:_BB0].rearrange("p (j q) -> p j q", j=2)
    bb_8 = p8_sb[0:1, _BB0:_O80].rearrange("p (k j q) -> p k j q", k=KC, j=2)
    o8_2 = p8_sb[0:1, _O80:_O80 + 2 * M].rearrange("p (j m) -> p j m", j=2)

    # ================= prologue =================
    # Btilde[h, m] = W1 @ q^2 + b1, stored fp8 twice (planes for DoubleRow
    # injection via the 0.5*I identity pair).
    btz = consts.tile([128, HC, 2, M], FP8, tag="btz")
    for hpair in range(2):
        bt_ps = ps2b.tile([128, 2, M], F32, tag="psx", name=f"bt{hpair}")
        for sub in range(2):
            hc = hpair * 2 + sub
            for i in range(DP):
                nc.tensor.matmul(bt_ps[:, sub, :],
                                 w1_8[:, i, :, hc * 128:(hc + 1) * 128],
                                 q2_8[:, i, :, :],
                                 start=(i == 0), stop=False, perf_mode=DR)
            nc.tensor.matmul(bt_ps[:, sub, :],
                             b1_bf[:, hc * 128:(hc + 1) * 128], onesb,
                             start=False, stop=True)
        for j in range(2):
            nc.scalar.activation(btz[:, hpair * 2:hpair * 2 + 2, j, :],
                                 bt_ps[:], AF.Copy)

    # fg mask broadcast to all partitions
    mask_ps = ps2b.tile([128, MC * NL], F32, tag="psx", name="maskp")
    nc.tensor.matmul(mask_ps[:], onesf, fg_row, start=True, stop=True)
    mask_bc = consts.tile([128, MC, NL], F32, tag="mask_bc")
    nc.vector.tensor_copy(mask_bc[:], mask_ps[:].rearrange(
        "p (c n) -> p c n", c=MC))

    # cost'[n, m] = ||Mn||^2 - 2 Mn.Qm  (x2 applied below; ||Q||^2 on host)
    g_t = psd.tile([128, MC, NL], F32, tag="psd", name="g_t")
    for mc in range(MC):
        for dc in range(DC):
            nc.tensor.matmul(g_t[:, mc, :],
                             qt_bf[:, dc, mc * 128:(mc + 1) * 128],
                             mt_bf[:, dc, :],
                             start=(dc == 0), stop=False)
        nc.tensor.matmul(g_t[:, mc, :], onesf, mn2h, start=False, stop=True)
    cost_t = consts.tile([128, MC, NL], F32, tag="cost_t")
    nc.vector.tensor_scalar_mul(cost_t[:], g_t[:], 2.0)

    d_ps = psd.tile([128, MC, NL], F32, tag="psd", name="d_ps")

    # ================= main loop over local rows =================
    MULT = mybir.AluOpType.mult
    ADD = mybir.AluOpType.add
    for n in range(NL):
        u = diffp.tile([128, DC, M], FP8, tag="u")
        for dc in range(DC):
            eng = nc.vector if U_SPLIT[dc] == "v" else nc.gpsimd
            eng.tensor_scalar(u[:, dc, :], qt_bf[:, dc, :],
                              s1f[:, dc, n:n + 1], s2f[:, dc, n:n + 1],
                              op0=MULT, op1=ADD)

        h1 = h1p.tile([128, HC, M], FP8, tag="h1")
        for hpair in range(2):
            p1 = ps2b.tile([128, 2, M], F32, tag="psx", name=f"p1_{hpair}")
            for sub in range(2):
                hc = hpair * 2 + sub
                for i in range(DP):
                    nc.tensor.matmul(p1[:, sub, :],
                                     w1_8[:, i, :, hc * 128:(hc + 1) * 128],
                                     u[:, 2 * i:2 * i + 2, :],
                                     start=(i == 0), stop=False, perf_mode=DR)
                nc.tensor.matmul(p1[:, sub, :], ih_8,
                                 btz[:, hc, :, :],
                                 start=False, stop=True, perf_mode=DR)
            nc.scalar.activation(h1[:, hpair * 2:hpair * 2 + 2, :], p1[:],
                                 AF.Lrelu, alpha=NEG_SLOPE)

        p2 = ps2b.tile([128, KC, M], F32, tag="psx", name="p2")
        for kc in range(KC):
            for i in range(HP):
                nc.tensor.matmul(p2[:, kc, :],
                                 w2_8[:, i, :, kc * 128:(kc + 1) * 128],
                                 h1[:, 2 * i:2 * i + 2, :],
                                 start=(i == 0), stop=False, perf_mode=DR)
            nc.tensor.matmul(p2[:, kc, :], bb_8[:, kc, :, :], o8_2,
                             start=False, stop=True, perf_mode=DR)
        h2 = h2p.tile([128, KC, M], FP8, tag="h2")
        nc.scalar.activation(h2[:], p2[:], AF.Lrelu, alpha=NEG_SLOPE)

        for mc in range(MC):
            nc.tensor.matmul(d_ps[:, mc, n:n + 1],
                             h2[:, :, mc * 128:(mc + 1) * 128], w3_8,
                             start=True, stop=True, perf_mode=DR)

    # ================= epilogue =================
    e_t = ep.tile([128, MC, NL], F32, tag="e_t")
    nc.scalar.activation(e_t[:], d_ps[:], AF.Exp, bias=b3n, scale=-1.0)
    w_t = ep.tile([128, MC, NL], F32, tag="w_t")
    nc.vector.tensor_mul(w_t[:], e_t[:], cost_t[:])
    efg_t = ep.tile([128, MC, NL], F32, tag="efg_t")
    nc.vector.tensor_mul(efg_t[:], e_t[:], mask_bc[:])
    wfg_t = ep.tile([128, MC, NL], F32, tag="wfg_t")
    nc.vector.tensor_mul(wfg_t[:], w_t[:], mask_bc[:])

    stats_sb = consts.tile([128, 4, MC], F32, tag="stats_sb")
    for s, src in ((0, e_t), (1, efg_t), (2, w_t), (3, wfg_t)):
        nc.vector.tensor_reduce(stats_sb[:, s, :], src[:],
                                axis=mybir.AxisListType.X,
                                op=mybir.AluOpType.add)
    nc.sync.dma_start(stats[:], stats_sb[:])


def build_program():
    nc = bacc.Bacc("TRN2", target_bir_lowering=False, debug=False,
                   num_devices=N_CORES)
    pf = nc.dram_tensor("pf", [128, _PF_COLS], F32, kind="ExternalInput").ap()
    pb = nc.dram_tensor("pb", [128, _PB_COLS], BF16, kind="ExternalInput").ap()
    p8 = nc.dram_tensor("p8", [128, _P8_COLS], FP8, kind="ExternalInput").ap()
    stats = nc.dram_tensor("stats", [128, 4, MC], F32,
                           kind="ExternalOutput").ap()

    with tile.TileContext(nc) as tc, ExitStack() as ctx:
        consts = ctx.enter_context(tc.tile_pool(name="consts", bufs=1))
        pf_sb = consts.tile([128, _PF_COLS], F32, tag="pf_sb")
        nc.sync.dma_start(pf_sb[:], pf[:])
        pb_sb = consts.tile([128, _PB_COLS], BF16, tag="pb_sb")
        nc.sync.dma_start(pb_sb[:], pb[:])
        p8_sb = consts.tile([128, _P8_COLS], FP8, tag="p8_sb")
        nc.sync.dma_start(p8_sb[:], p8[:])
        emit_body(nc, tc, ctx, pf_sb, pb_sb, p8_sb, stats)

    nc.compile()
    return nc


def shard_inputs(M_aug, Q, W1, b1, W2, b2, W3, b3, N_fg):
    """Host-side layout prep. Returns per-core input maps."""
    import ml_dtypes
    f = np.float32
    bf = ml_dtypes.bfloat16
    f8 = ml_dtypes.float8_e4m3
    M_aug = np.asarray(M_aug, f)
    Q = np.asarray(Q, f)
    W1 = np.asarray(W1, f)
    W2 = np.asarray(W2, f)
    W3 = np.asarray(W3, f)
    b1 = np.asarray(b1, f)
    b2 = np.asarray(b2, f)
    b3 = np.asarray(b3, f)
    nfg = int(N_fg)

    def part_major(a2d, chunks):  # [C*128, F] -> [128, C*F]
        cdim, fdim = a2d.shape
        assert cdim == chunks * 128
        return np.ascontiguousarray(
            a2d.reshape(chunks, 128, fdim).transpose(1, 0, 2)).reshape(128, -1)

    def dr_layout(a2d, pairs):  # [2*pairs*128, F] -> [128, pairs*2*F]
        return part_major(a2d, 2 * pairs)

    # ---- fp8 packed tensor (shared across cores) ----
    p8_v = np.zeros((128, _P8_COLS), f8)
    p8_v[:, _W10:_W20] = dr_layout(W1.T, DP).astype(f8)
    p8_v[:, _W20:_W30] = dr_layout(W2.T, HP).astype(f8)
    p8_v[:, _W30:_W30 + 2] = dr_layout(W3.reshape(K2, 1), 1).astype(f8)
    qt_bf_full = Q.T.astype(bf)
    q2 = (qt_bf_full.astype(f) ** 2).astype(bf).astype(f)
    p8_v[:, _Q20:_IH0] = dr_layout(q2, DP).astype(f8)
    ih = np.zeros((2, 128, 128), f)
    ih[0] = 0.5 * np.eye(128)
    ih[1] = 0.5 * np.eye(128)
    # identity pair layout [128, 2, 128]: partition p, plane j, free q
    p8_v[:, _IH0:_BB0] = ih.transpose(1, 0, 2).reshape(128, -1).astype(f8)
    bb = np.zeros((KC, 2, 128), f)
    bb[:, 0, :] = 0.5 * b2.reshape(KC, 128)
    bb[:, 1, :] = 0.5 * b2.reshape(KC, 128)
    p8_v[0, _BB0:_O80] = bb.reshape(-1).astype(f8)
    p8_v[0, _O80:_O80 + 2 * M] = np.ones(2 * M, f).astype(f8)
    p8_v = np.ascontiguousarray(p8_v)

    # ---- bf16 packed tensor (Q parts shared; M parts per core) ----
    pb_base = np.zeros((128, _PB_COLS), bf)
    pb_base[:, _QT0:_MT0] = part_major(Q.T, DC).astype(bf)
    pb_base[0, _B10:_B10 + H] = b1.astype(bf)
    pb_base[0, _OB0:_OB0 + M] = np.ones(M, f).astype(bf)

    # ---- f32 packed tensor (per core) ----
    pf_base = np.zeros((128, _PF_COLS), f)
    pf_base[0, _ON0:_ON0 + 128] = 1.0
    pf_base[:, _B30] = -float(b3[0])

    in_maps = []
    for c in range(N_CORES):
        rows = slice(c * NL, (c + 1) * NL)
        Mrows = M_aug[rows]
        pf_v = pf_base.copy()
        pf_v[:, _S10:_S20] = part_major(-2.0 * Mrows.T, DC)
        pf_v[:, _S20:_MN0] = part_major(Mrows.T ** 2, DC)
        pf_v[0, _MN0:_MN0 + NL] = 0.5 * (Mrows ** 2).sum(-1)
        gidx = np.arange(c * NL, (c + 1) * NL)
        pf_v[0, _FG0:_FG0 + MC * NL] = np.tile((gidx < nfg).astype(f), MC)
        pb_v = pb_base.copy()
        pb_v[:, _MT0:_B10] = part_major(-Mrows.T, DC).astype(bf)
        in_maps.append({"pf": np.ascontiguousarray(pf_v),
                        "pb": np.ascontiguousarray(pb_v),
                        "p8": p8_v})
    return in_maps


def combine(stats_list, Q):
    """stats_list: per-core [128, 4, MC] arrays -> (score, score_fg)."""
    st = np.stack([
        np.asarray(s, np.float64).transpose(1, 2, 0).reshape(4, M)
        for s in stats_list
    ])  # [C, 4, M]
    S1 = st[:, 0].sum(0)
    S1fg = st[:, 1].sum(0)
    Sc = st[:, 2].sum(0)
    Scfg = st[:, 3].sum(0)
    qn2 = (np.asarray(Q, np.float64) ** 2).sum(-1)
    score = Sc / S1 + qn2
    score_fg = Scfg / S1 + qn2 * (S1fg / S1)
    return score.astype(np.float32), score_fg.astype(np.float32)


_PROGRAM_CACHE = {}


def run(trace=False, **inputs):
    if "prog" not in _PROGRAM_CACHE:
        _PROGRAM_CACHE["prog"] = build_program()
    nc = _PROGRAM_CACHE["prog"]
    in_maps = shard_inputs(**inputs)
    res = run_bass_kernel_spmd(nc, in_maps, list(range(N_CORES)), trace=trace)
    outs = combine([res.results[c]["stats"] for c in range(N_CORES)],
                   inputs["Q"])
    return outs, res


def kernel(**inputs):
    outs, _ = run(trace=False, **inputs)
    return outs


# revision 3
# speedup vs baseline: 2.5492x; 1.2789x over previous
"""Trainium2 Bass kernel for the CT-metrics pairwise MLP scorer.

Computes, for M_aug [N,D] and Q [M,D] (N=M=512, D=1024):
    diff2[n,m,:] = (M_aug[n]-Q[m])**2
    cost[n,m]    = diff2.sum(-1)
    d[n,m]       = -(MLP(diff2[n,m,:]) + b3)      (D->512->256->1, leaky relu)
    bw           = softmax(d, axis=0)
    score[m]     = sum_n cost*bw,  score_fg[m] = sum_{n<N_fg} cost*bw

Sharding: N axis split across 8 cores (64 rows each). Each core computes
partial column sums S1 = sum_n exp(d), S1fg, Sc' = sum_n exp(d)*(||Mn||^2
- 2 M.Q^T) and Scfg' (flash-softmax; logits are O(1) so no max shift).
Host combine:  score = Sc'/S1 + ||Q||^2, score_fg = Scfg'/S1 + ||Q||^2*S1fg/S1.

Device algorithm (fp8 DoubleRow formulation):
  diff2 = q^2 - 2mq + m^2.  The n-independent W1 @ q^2 term is folded into
  a precomputed Btilde[h,m] = W1 @ q^2 + b1 (computed once on device from
  host-packed q^2, stored fp8, injected into each row's L1 PSUM via an
  identity-pair DoubleRow matmul).  Per row only
      u[d,m] = q[d,m]*(-2 m[n,d]) + m[n,d]^2
  is materialized - a single DVE/GPSIMD tensor_scalar op (bf16 in, f32
  scalars, fp8 out; DVE runs it in 2x_2p mode).  All three MLP layers are
  fp8e4 DoubleRow matmuls (0.5 cyc/row, 256-contraction per instruction);
  b2 is injected as an fp8 rank-2 DoubleRow matmul; b1 lives in Btilde, so
  the h1/h2 activations are bias-free and read multi-bank PSUM pairs in
  single ACT instructions.  Engine balance per row: ACT ~3.1us (h1+h2),
  PE ~2.7us (matmuls), DVE ~2.0us (6 u-chunks), GPSIMD ~1.6us (2 u-chunks).
"""

from contextlib import ExitStack

import numpy as np

import concourse.bass as bass
import concourse.tile as tile
from concourse import bacc, mybir
from concourse.bass_utils import run_bass_kernel_spmd

N_CORES = 8
N, M, D, H = 512, 512, 1024, 512
K2 = H // 2  # 256
NL = N // N_CORES  # 64 rows per core
DC, HC, KC, MC = D // 128, H // 128, K2 // 128, M // 128  # 8, 4, 2, 4
DP, HP = DC // 2, HC // 2  # DoubleRow plane-pairs: 4, 2
NEG_SLOPE = 0.01

F32 = mybir.dt.float32
BF16 = mybir.dt.bfloat16
FP8 = mybir.dt.float8e4
AF = mybir.ActivationFunctionType
DR = mybir.MatmulPerfMode.DoubleRow

# which engine produces each u chunk: 'v' = DVE, 'p' = GPSIMD
U_SPLIT = "vvpvvpvv"

# ---- packed f32 tensor column offsets ----
_S10 = 0                       # -2*M^T  [128, DC*NL]
_S20 = _S10 + DC * NL          # (M^T)^2 [128, DC*NL]
_MN0 = _S20 + DC * NL          # 0.5*||Mn||^2 row (partition 0) [1, NL]
_FG0 = _MN0 + NL               # fg mask row tiled MC times (part 0)
_ON0 = _FG0 + MC * NL          # ones row f32 (part 0) [1, 128]
_B30 = _ON0 + 128              # -b3 column [128, 1]
_PF_COLS = _B30 + 1

# ---- packed bf16 tensor column offsets ----
_QT0 = 0                       # Q^T chunks [128, DC*M]
_MT0 = _QT0 + DC * M           # -M^T chunks [128, DC*NL]
_B10 = _MT0 + DC * NL          # b1 row (part 0) [1, H]
_OB0 = _B10 + H                # ones row bf16 (part 0) [1, M]
_PB_COLS = _OB0 + M

# ---- packed fp8 tensor column offsets ----
_W10 = 0                       # W1 DoubleRow layout [128, DP*2*H]
_W20 = _W10 + DP * 2 * H       # W2 DoubleRow layout [128, HP*2*K2]
_W30 = _W20 + HP * 2 * K2      # W3 DoubleRow layout [128, 2*1]
_Q20 = _W30 + 2                # (Q^T)^2 DoubleRow layout [128, DP*2*M]
_IH0 = _Q20 + DP * 2 * M       # 0.5*I identity pair [128, 2*128]
_BB0 = _IH0 + 2 * 128          # b2/2 pairs (part 0) [1, KC*2*128]
_O80 = _BB0 + KC * 2 * 128     # ones pair fp8 (part 0) [1, 2*M]
_P8_COLS = _O80 + 2 * M


def emit_body(nc, tc, ctx, pf_sb, pb_sb, p8_sb, stats):
    def pool(name, bufs, space="SBUF"):
        return ctx.enter_context(tc.tile_pool(name=name, bufs=bufs, space=space))

    consts = ctx.enter_context(tc.tile_pool(name="consts2", bufs=1))
    diffp = pool("diffp", 2)
    h1p = pool("h1p", 2)
    h2p = pool("h2p", 2)
    ep = pool("ep", 1)
    # one shared 2-bank PSUM pool: per row holds l1a, l1b, p2 (rotating)
    ps2b = pool("ps2b", 3, "PSUM")
    psd = pool("psd", 1, "PSUM")  # [128, MC, NL] logits / cost psum

    s1f = pf_sb[:, _S10:_S20].rearrange("p (c n) -> p c n", c=DC)
    s2f = pf_sb[:, _S20:_MN0].rearrange("p (c n) -> p c n", c=DC)
    mn2h = pf_sb[0:1, _MN0:_MN0 + NL]
    fg_row = pf_sb[0:1, _FG0:_FG0 + MC * NL]
    onesf = pf_sb[0:1, _ON0:_ON0 + 128]
    b3n = pf_sb[:, _B30:_B30 + 1]

    qt_bf = pb_sb[:, _QT0:_MT0].rearrange("p (c m) -> p c m", c=DC)
    mt_bf = pb_sb[:, _MT0:_B10].rearrange("p (c n) -> p c n", c=DC)
    b1_bf = pb_sb[0:1, _B10:_B10 + H]
    onesb = pb_sb[0:1, _OB0:_OB0 + M]

    w1_8 = p8_sb[:, _W10:_W20].rearrange("p (i j h) -> p i j h", i=DP, j=2)
    w2_8 = p8_sb[:, _W20:_W30].rearrange("p (i j k) -> p i j k", i=HP, j=2)
    w3_8 = p8_sb[:, _W30:_W30 + 2].rearrange("p (j o) -> p j o", j=2)
    q2_8 = p8_sb[:, _Q20:_IH0].rearrange("p (i j m) -> p i j m", i=DP, j=2)
    ih_8 = p8_sb[:, _IH0:_BB0].rearrange("p (j q) -> p j q", j=2)
    bb_8 = p8_sb[0:1, _BB0:_O80].rearrange("p (k j q) -> p k j q", k=KC, j=2)
    o8_2 = p8_sb[0:1, _O80:_O80 + 2 * M].rearrange("p (j m) -> p j m", j=2)

    # ================= prologue =================
    # Btilde[h, m] = W1 @ q^2 + b1, stored fp8 twice (planes for DoubleRow
    # injection via the 0.5*I identity pair).
    btz = consts.tile([128, HC, 2, M], FP8, tag="btz")
    for hpair in range(2):
        bt_ps = ps2b.tile([128, 2, M], F32, tag="psx", name=f"bt{hpair}")
        for sub in range(2):
            hc = hpair * 2 + sub
            for i in range(DP):
                nc.tensor.matmul(bt_ps[:, sub, :],
                                 w1_8[:, i, :, hc * 128:(hc + 1) * 128],
                                 q2_8[:, i, :, :],
                                 start=(i == 0), stop=False, perf_mode=DR)
            nc.tensor.matmul(bt_ps[:, sub, :],
                             b1_bf[:, hc * 128:(hc + 1) * 128], onesb,
                             start=False, stop=True)
        for j in range(2):
            nc.scalar.activation(btz[:, hpair * 2:hpair * 2 + 2, j, :],
                                 bt_ps[:], AF.Copy)

    # fg mask broadcast to all partitions
    mask_ps = ps2b.tile([128, MC * NL], F32, tag="psx", name="maskp")
    nc.tensor.matmul(mask_ps[:], onesf, fg_row, start=True, stop=True)
    mask_bc = consts.tile([128, MC, NL], F32, tag="mask_bc")
    nc.vector.tensor_copy(mask_bc[:], mask_ps[:].rearrange(
        "p (c n) -> p c n", c=MC))

    # cost'[n, m] = ||Mn||^2 - 2 Mn.Qm  (x2 applied below; ||Q||^2 on host)
    g_t = psd.tile([128, MC, NL], F32, tag="psd", name="g_t")
    for mc in range(MC):
        for dc in range(DC):
            nc.tensor.matmul(g_t[:, mc, :],
                             qt_bf[:, dc, mc * 128:(mc + 1) * 128],
                             mt_bf[:, dc, :],
                             start=(dc == 0), stop=False)
        nc.tensor.matmul(g_t[:, mc, :], onesf, mn2h, start=False, stop=True)
    cost_t = consts.tile([128, MC, NL], F32, tag="cost_t")
    nc.vector.tensor_scalar_mul(cost_t[:], g_t[:], 2.0)

    d_ps = psd.tile([128, MC, NL], F32, tag="psd", name="d_ps")

    # ================= main loop over local rows =================
    # Software-pipelined so the in-order PE stream never waits on ACT:
    # iteration k emits  u(k), L1(k), h1(k), L2(k-1), h2(k-1), L3(k-2).
    MULT = mybir.AluOpType.mult
    ADD = mybir.AluOpType.add
    h1_hist = {}
    h2_hist = {}

    def emit_u_l1_h1(n):
        u = diffp.tile([128, DC, M], FP8, tag="u")
        for dc in range(DC):
            eng = nc.vector if U_SPLIT[dc] == "v" else nc.gpsimd
            eng.tensor_scalar(u[:, dc, :], qt_bf[:, dc, :],
                              s1f[:, dc, n:n + 1], s2f[:, dc, n:n + 1],
                              op0=MULT, op1=ADD)
        h1 = h1p.tile([128, HC, M], FP8, tag="h1")
        h1_hist[n] = h1
        for hpair in range(2):
            p1 = ps2b.tile([128, 2, M], F32, tag="psx", name=f"p1_{hpair}")
            for sub in range(2):
                hc = hpair * 2 + sub
                for i in range(DP):
                    nc.tensor.matmul(p1[:, sub, :],
                                     w1_8[:, i, :, hc * 128:(hc + 1) * 128],
                                     u[:, 2 * i:2 * i + 2, :],
                                     start=(i == 0), stop=False, perf_mode=DR)
                nc.tensor.matmul(p1[:, sub, :], ih_8,
                                 btz[:, hc, :, :],
                                 start=False, stop=True, perf_mode=DR)
            nc.scalar.activation(h1[:, hpair * 2:hpair * 2 + 2, :], p1[:],
                                 AF.Lrelu, alpha=NEG_SLOPE)

    def emit_l2_h2(n):
        h1 = h1_hist.pop(n)
        p2 = ps2b.tile([128, KC, M], F32, tag="psx", name="p2")
        for kc in range(KC):
            for i in range(HP):
                nc.tensor.matmul(p2[:, kc, :],
                                 w2_8[:, i, :, kc * 128:(kc + 1) * 128],
                                 h1[:, 2 * i:2 * i + 2, :],
                                 start=(i == 0), stop=False, perf_mode=DR)
            nc.tensor.matmul(p2[:, kc, :], bb_8[:, kc, :, :], o8_2,
                             start=False, stop=True, perf_mode=DR)
        h2 = h2p.tile([128, KC, M], FP8, tag="h2")
        h2_hist[n] = h2
        nc.scalar.activation(h2[:], p2[:], AF.Lrelu, alpha=NEG_SLOPE)

    def emit_l3(n):
        h2 = h2_hist.pop(n)
        for mc in range(MC):
            nc.tensor.matmul(d_ps[:, mc, n:n + 1],
                             h2[:, :, mc * 128:(mc + 1) * 128], w3_8,
                             start=True, stop=True, perf_mode=DR)

    for n in range(NL):
        emit_u_l1_h1(n)
        if n >= 1:
            emit_l2_h2(n - 1)
        if n >= 2:
            emit_l3(n - 2)
    emit_l2_h2(NL - 1)
    emit_l3(NL - 2)
    emit_l3(NL - 1)

    # ================= epilogue =================
    e_t = ep.tile([128, MC, NL], F32, tag="e_t")
    nc.scalar.activation(e_t[:], d_ps[:], AF.Exp, bias=b3n, scale=-1.0)
    w_t = ep.tile([128, MC, NL], F32, tag="w_t")
    nc.vector.tensor_mul(w_t[:], e_t[:], cost_t[:])
    efg_t = ep.tile([128, MC, NL], F32, tag="efg_t")
    nc.vector.tensor_mul(efg_t[:], e_t[:], mask_bc[:])
    wfg_t = ep.tile([128, MC, NL], F32, tag="wfg_t")
    nc.vector.tensor_mul(wfg_t[:], w_t[:], mask_bc[:])

    stats_sb = consts.tile([128, 4, MC], F32, tag="stats_sb")
    for s, src in ((0, e_t), (1, efg_t), (2, w_t), (3, wfg_t)):
        nc.vector.tensor_reduce(stats_sb[:, s, :], src[:],
                                axis=mybir.AxisListType.X,
                                op=mybir.AluOpType.add)
    nc.sync.dma_start(stats[:], stats_sb[:])


def build_program():
    nc = bacc.Bacc("TRN2", target_bir_lowering=False, debug=False,
                   num_devices=N_CORES)
    pf = nc.dram_tensor("pf", [128, _PF_COLS], F32, kind="ExternalInput").ap()
    pb = nc.dram_tensor("pb", [128, _PB_COLS], BF16, kind="ExternalInput").ap()
    p8 = nc.dram_tensor("p8", [128, _P8_COLS], FP8, kind="ExternalInput").ap()
    stats = nc.dram_tensor("stats", [128, 4, MC], F32,
                           kind="ExternalOutput").ap()

    with tile.TileContext(nc) as tc, ExitStack() as ctx:
        consts = ctx.enter_context(tc.tile_pool(name="consts", bufs=1))
        pf_sb = consts.tile([128, _PF_COLS], F32, tag="pf_sb")
        nc.sync.dma_start(pf_sb[:], pf[:])
        pb_sb = consts.tile([128, _PB_COLS], BF16, tag="pb_sb")
        nc.sync.dma_start(pb_sb[:], pb[:])
        p8_sb = consts.tile([128, _P8_COLS], FP8, tag="p8_sb")
        nc.sync.dma_start(p8_sb[:], p8[:])
        emit_body(nc, tc, ctx, pf_sb, pb_sb, p8_sb, stats)

    nc.compile()
    return nc


def shard_inputs(M_aug, Q, W1, b1, W2, b2, W3, b3, N_fg):
    """Host-side layout prep. Returns per-core input maps."""
    import ml_dtypes
    f = np.float32
    bf = ml_dtypes.bfloat16
    f8 = ml_dtypes.float8_e4m3
    M_aug = np.asarray(M_aug, f)
    Q = np.asarray(Q, f)
    W1 = np.asarray(W1, f)
    W2 = np.asarray(W2, f)
    W3 = np.asarray(W3, f)
    b1 = np.asarray(b1, f)
    b2 = np.asarray(b2, f)
    b3 = np.asarray(b3, f)
    nfg = int(N_fg)

    def part_major(a2d, chunks):  # [C*128, F] -> [128, C*F]
        cdim, fdim = a2d.shape
        assert cdim == chunks * 128
        return np.ascontiguousarray(
            a2d.reshape(chunks, 128, fdim).transpose(1, 0, 2)).reshape(128, -1)

    def dr_layout(a2d, pairs):  # [2*pairs*128, F] -> [128, pairs*2*F]
        return part_major(a2d, 2 * pairs)

    # ---- fp8 packed tensor (shared across cores) ----
    p8_v = np.zeros((128, _P8_COLS), f8)
    p8_v[:, _W10:_W20] = dr_layout(W1.T, DP).astype(f8)
    p8_v[:, _W20:_W30] = dr_layout(W2.T, HP).astype(f8)
    p8_v[:, _W30:_W30 + 2] = dr_layout(W3.reshape(K2, 1), 1).astype(f8)
    qt_bf_full = Q.T.astype(bf)
    q2 = (qt_bf_full.astype(f) ** 2).astype(bf).astype(f)
    p8_v[:, _Q20:_IH0] = dr_layout(q2, DP).astype(f8)
    ih = np.zeros((2, 128, 128), f)
    ih[0] = 0.5 * np.eye(128)
    ih[1] = 0.5 * np.eye(128)
    # identity pair layout [128, 2, 128]: partition p, plane j, free q
    p8_v[:, _IH0:_BB0] = ih.transpose(1, 0, 2).reshape(128, -1).astype(f8)
    bb = np.zeros((KC, 2, 128), f)
    bb[:, 0, :] = 0.5 * b2.reshape(KC, 128)
    bb[:, 1, :] = 0.5 * b2.reshape(KC, 128)
    p8_v[0, _BB0:_O80] = bb.reshape(-1).astype(f8)
    p8_v[0, _O80:_O80 + 2 * M] = np.ones(2 * M, f).astype(f8)
    p8_v = np.ascontiguousarray(p8_v)

    # ---- bf16 packed tensor (Q parts shared; M parts per core) ----
    pb_base = np.zeros((128, _PB_COLS), bf)
    pb_base[:, _QT0:_MT0] = part_major(Q.T, DC).astype(bf)
    pb_base[0, _B10:_B10 + H] = b1.astype(bf)
    pb_base[0, _OB0:_OB0 + M] = np.ones(M, f).astype(bf)

    # ---- f32 packed tensor (per core) ----
    pf_base = np.zeros((128, _PF_COLS), f)
    pf_base[0, _ON0:_ON0 + 128] = 1.0
    pf_base[:, _B30] = -float(b3[0])

    in_maps = []
    for c in range(N_CORES):
        rows = slice(c * NL, (c + 1) * NL)
        Mrows = M_aug[rows]
        pf_v = pf_base.copy()
        pf_v[:, _S10:_S20] = part_major(-2.0 * Mrows.T, DC)
        pf_v[:, _S20:_MN0] = part_major(Mrows.T ** 2, DC)
        pf_v[0, _MN0:_MN0 + NL] = 0.5 * (Mrows ** 2).sum(-1)
        gidx = np.arange(c * NL, (c + 1) * NL)
        pf_v[0, _FG0:_FG0 + MC * NL] = np.tile((gidx < nfg).astype(f), MC)
        pb_v = pb_base.copy()
        pb_v[:, _MT0:_B10] = part_major(-Mrows.T, DC).astype(bf)
        in_maps.append({"pf": np.ascontiguousarray(pf_v),
                        "pb": np.ascontiguousarray(pb_v),
                        "p8": p8_v})
    return in_maps


def combine(stats_list, Q):
    """stats_list: per-core [128, 4, MC] arrays -> (score, score_fg)."""
    st = np.stack([
        np.asarray(s, np.float64).transpose(1, 2, 0).reshape(4, M)
        for s in stats_list
    ])  # [C, 4, M]
    S1 = st[:, 0].sum(0)
    S1fg = st[:, 1].sum(0)
    Sc = st[:, 2].sum(0)
    Scfg = st[:, 3].sum(0)
    qn2 = (np.asarray(Q, np.float64) ** 2).sum(-1)
    score = Sc / S1 + qn2
    score_fg = Scfg / S1 + qn2 * (S1fg / S1)
    return score.astype(np.float32), score_fg.astype(np.float32)


_PROGRAM_CACHE = {}


def run(trace=False, **inputs):
    if "prog" not in _PROGRAM_CACHE:
        _PROGRAM_CACHE["prog"] = build_program()
    nc = _PROGRAM_CACHE["prog"]
    in_maps = shard_inputs(**inputs)
    res = run_bass_kernel_spmd(nc, in_maps, list(range(N_CORES)), trace=trace)
    outs = combine([res.results[c]["stats"] for c in range(N_CORES)],
                   inputs["Q"])
    return outs, res


def kernel(**inputs):
    outs, _ = run(trace=False, **inputs)
    return outs
